# revision 13
# baseline (speedup 1.0000x reference)
"""Adaptive embedding (4-cluster masked embedding + projection) on 8 trn2 cores.

Sharding: tokens are globally sorted by gather-bucket and dealt round-robin to
the 8 NeuronCores, so per-core per-bucket counts are equal (+-1) and tile
padding is minimal; the embedding/projection tables are replicated.

Host does ROUTING only; the device gathers rows with ONE `dma_gather` per
bucket (a bucket is a cluster, or a greedily-sized vocab range of clusters 2/3
- dma_gather indices are int16, so a range spans at most 32000 rows and at
most 2048 global tokens = 2 tiles per core). Clusters 1-3 gather with
transpose=True: the XBAR delivers the rows dim-on-partition / token-on-free,
which IS the stationary matmul operand - no PE transpose, no PSUM cast
round-trip. Cluster 0 gathers un-transposed and streams straight to DRAM.

Cluster 2/3 table rows are zero-padded to 256 B (dma_gather element
granularity); the padded dims multiply zero weight rows, contributing 0.

The sqrt(D_PROJ)=32 output scale is an exact power of two, folded into the
tables / projection matrices before the bf16 cast. Output is written bf16 in
a partition-major layout the host un-transposes and upcasts.

Numerics: device data path is bf16 (inputs rounded once on host); matmul
accumulation is fp32 in PSUM. Worst-case elementwise error ~2^-8 relative,
far inside the 2e-2 gate.
"""

import os

import numpy as np

CUTOFFS = (0, 20000, 40000, 200000, 267735)
D_PROJ = 1024
N_CORES = 8
P = 128
RNG = 32000  # max vocab rows per gather bucket (int16 index headroom)
TCAP = 2    # target tiles per core per range bucket

VOCABS = (20000, 20000, 160000, 67735)
ELEMS = (1024, 256, 128, 128)  # gather row widths; clusters 2/3 pad to 256B

_BUILD_CACHE = {}
LAST_RESULT = None  # BassKernelResults of the most recent run (for profiling)


def _cfg():
    return dict(
        evac=os.environ.get("KERNEL_EVAC", "vs"),
        warm_mm=int(os.environ.get("KERNEL_WARM_MM", "0")),
        store_batch=int(os.environ.get("KERNEL_STORE_BATCH", "2")),
        nq=int(os.environ.get("KERNEL_NQ", "4")),
    )


def _build(bdefs, caps, cfg):
    """SPMD Bass program. bdefs: tuple of (cluster, vocab_lo, vocab_hi);
    caps: tiles per bucket (identical on every core)."""
    import concourse.bacc as bacc
    import concourse.tile as tile
    from concourse import mybir

    f32 = mybir.dt.float32
    bf16 = mybir.dt.bfloat16
    i16 = mybir.dt.int16
    nb = len(bdefs)

    # per-cluster totals and per-bucket tile offsets within the cluster
    ctiles = [0] * 4
    tbase = []
    for b, (cl, lo, hi) in enumerate(bdefs):
        tbase.append(ctiles[cl])
        ctiles[cl] += caps[b]
    icols = np.concatenate([[0], np.cumsum([caps[b] * 8 for b in range(nb)])])

    nc = bacc.Bacc(
        "TRN2", target_bir_lowering=False, num_swdge_queues=cfg["nq"]
    )
    emb = [
        nc.dram_tensor(f"emb{i}", [VOCABS[i], ELEMS[i]], bf16, kind="ExternalInput")
        for i in range(4)
    ]
    # weights blob [128, 4096]: W1[0:128] | W1[128:256] | W2z | W3z
    wts = nc.dram_tensor("wts", [P, 4 * D_PROJ], bf16, kind="ExternalInput")
    idx16 = nc.dram_tensor("idx16", [P, int(icols[-1])], i16, kind="ExternalInput")
    out = [
        nc.dram_tensor(f"out{i}", [P, ctiles[i], D_PROJ], bf16, kind="ExternalOutput")
        for i in range(4)
    ]

    with tile.TileContext(nc) as tc:
        with (
            tc.tile_pool(name="const", bufs=1) as cpool,
            tc.tile_pool(name="mpsum", bufs=3, space="PSUM") as mpool,
        ):
            idxt = cpool.tile([P, int(icols[-1])], i16, name="idxt")
            nc.sync.dma_start(out=idxt[:], in_=idx16[:])

            # gather tiles: cluster 0 token-major [128, T, 1024]; clusters 1-3
            # transposed [128, dim_chunks, T*128] (dim on partition)
            g = []
            for b, (cl, lo, hi) in enumerate(bdefs):
                if cl == 0:
                    t = cpool.tile([P, caps[b], ELEMS[0]], bf16, name=f"g{b}")
                else:
                    t = cpool.tile(
                        [P, ELEMS[cl] // P, caps[b] * P], bf16, name=f"g{b}"
                    )
                g.append(t)

            qn = [0]

            def gather(b):
                cl, lo, hi = bdefs[b]
                n = caps[b] * P
                nc.gpsimd.dma_gather(
                    out_ap=g[b][:, :, :],
                    in_ap=emb[cl][lo:hi, :],
                    idxs_ap=idxt[:, int(icols[b]) : int(icols[b + 1])],
                    num_idxs=n,
                    num_idxs_reg=n,
                    elem_size=ELEMS[cl],
                    transpose=(cl != 0),
                    queue_num=qn[0] % cfg["nq"],
                )
                qn[0] += 1

            # issue order: cluster-2 ranges first (longest PE chain), cluster 0
            # early for store flow, then the rest
            c2b = [b for b in range(nb) if bdefs[b][0] == 2]
            c3b = [b for b in range(nb) if bdefs[b][0] == 3]
            gorder = c2b[:2] + [0] + c2b[2:] + [1] + c3b
            for b in gorder:
                gather(b)

            # cluster 0: straight copy to DRAM (p-major layout = same layout)
            nc.sync.dma_start(out=out[0][:, :, :], in_=g[0][:, :, :])

            wt = cpool.tile([P, 4 * D_PROJ], bf16, name="wt")
            nc.scalar.dma_start(out=wt[:], in_=wts[:])

            # optional PE warm-up (HAM clock-gate) while gathers run
            nwarm = cfg["warm_mm"]
            if nwarm:
                warm = cpool.tile([P, 512 + P], bf16, name="warm")
                nc.gpsimd.memset(warm[:], 0.0)
                wps = mpool.tile([P, D_PROJ], f32, tag="ps", name="warm_ps")
                for w in range(nwarm):
                    nc.tensor.matmul(
                        wps[:, 0:512], warm[:, 512 : 512 + P], warm[:, 0:512],
                        start=True, stop=True,
                    )

            stage = {
                i: cpool.tile([P, ctiles[i], D_PROJ], bf16, name=f"stage{i}")
                for i in (1, 2, 3)
            }

            evac_pat = cfg["evac"]
            evac_state = [0]

            def evac(i, t, ps):
                e = evac_pat[evac_state[0] % len(evac_pat)]
                evac_state[0] += 1
                dst = stage[i][:, t, :]
                if e == "v":
                    nc.vector.tensor_copy(out=dst, in_=ps[:])
                elif e == "s":
                    nc.scalar.copy(out=dst, in_=ps[:])
                else:  # split across both engines
                    nc.vector.tensor_copy(out=dst[:, 0:512], in_=ps[:, 0:512])
                    nc.scalar.copy(out=dst[:, 512:1024], in_=ps[:, 512:1024])

            def store(i, t0, t1):
                nc.sync.dma_start(
                    out=out[i][:, t0:t1, :], in_=stage[i][:, t0:t1, :]
                )

            sb = cfg["store_batch"]
            pend = {1: 0, 2: 0, 3: 0}

            def finish_tile(i, gt, ps):
                evac(i, gt, ps)
                if gt + 1 - pend[i] >= sb or gt == ctiles[i] - 1:
                    store(i, pend[i], gt + 1)
                    pend[i] = gt + 1

            # K-chunks per cluster and weight column base
            #  c1: 2 chunks of 128 (cols 0/1024); c2: 1 chunk (cols 2048, rows
            #  64+ of W2z zero); c3: 1 chunk (cols 3072, rows 16+ zero)
            def project(i, b):
                nch = 2 if i == 1 else 1
                wcol = {1: 0, 2: 2 * D_PROJ, 3: 3 * D_PROJ}[i]
                for t in range(caps[b]):
                    gt = tbase[b] + t
                    ps = mpool.tile([P, D_PROJ], f32, tag="ps", name=f"ps{i}_{gt}")
                    for k in range(nch):
                        lhsT = g[b][:, k, t * P : (t + 1) * P]
                        for n in range(2):
                            col = wcol + k * D_PROJ + n * 512
                            nc.tensor.matmul(
                                ps[:, n * 512 : (n + 1) * 512],
                                lhsT,
                                wt[:, col : col + 512],
                                start=(k == 0),
                                stop=(k == nch - 1),
                            )
                    finish_tile(i, gt, ps)

            for b in c2b:
                project(2, b)
            project(1, 1)
            for b in c3b:
                project(3, b)

    nc.compile()
    return nc


def _greedy_ranges(sorted_locs, vocab):
    """Cut [0, vocab) so each range has <= 8*128*TCAP tokens and <= RNG rows."""
    cut_tok = N_CORES * P * TCAP
    bounds = []
    lo, pos = 0, 0
    while lo < vocab:
        hi = lo + RNG
        if pos + cut_tok < len(sorted_locs):
            hi = min(hi, int(sorted_locs[pos + cut_tok]))
        hi = min(hi, vocab)
        if hi <= lo:
            hi = lo + 1
        bounds.append((lo, hi))
        pos = int(np.searchsorted(sorted_locs, hi))
        lo = hi
    return bounds


def kernel(tokens, emb0, emb1, emb2, emb3, proj1, proj2, proj3):
    global LAST_RESULT
    import ml_dtypes
    from concourse.bass_utils import run_bass_kernel_spmd

    bf16 = ml_dtypes.bfloat16
    toks = np.asarray(tokens).astype(np.int64, copy=False)
    nb_, ns = toks.shape
    assert nb_ == N_CORES and ns % P == 0

    # fold sqrt(1024)=32 (exact in bf16) and round tables once on the host;
    # clusters 2/3 rows padded to 256B with zeros
    scale = np.float32(32.0)
    e0 = (np.asarray(emb0, dtype=np.float32) * scale).astype(bf16)
    e1 = np.asarray(emb1, dtype=np.float32).astype(bf16)
    e2 = np.zeros((VOCABS[2], 128), bf16)
    e2[:, :64] = np.asarray(emb2, dtype=np.float32).astype(bf16)
    e3 = np.zeros((VOCABS[3], 128), bf16)
    e3[:, :16] = np.asarray(emb3, dtype=np.float32).astype(bf16)
    embs = [np.ascontiguousarray(e) for e in (e0, e1, e2, e3)]

    w1 = (np.asarray(proj1, dtype=np.float32) * scale).astype(bf16)
    w2z = np.zeros((P, D_PROJ), bf16)
    w2z[:64] = (np.asarray(proj2, dtype=np.float32) * scale).astype(bf16)
    w3z = np.zeros((P, D_PROJ), bf16)
    w3z[:16] = (np.asarray(proj3, dtype=np.float32) * scale).astype(bf16)
    wts = np.ascontiguousarray(
        np.concatenate([w1[0:P], w1[P : 2 * P], w2z, w3z], axis=1)
    )

    # ---- global routing: bucket per token, global sort, round-robin to cores
    ft = toks.reshape(-1)
    cuts = np.asarray(CUTOFFS, dtype=np.int64)
    fcl = np.searchsorted(cuts[1:-1], ft, side="right")
    sizes = np.asarray(VOCABS, dtype=np.int64)
    floc = np.clip(ft - cuts[fcl], 0, sizes[fcl] - 1)

    b2 = _greedy_ranges(np.sort(floc[fcl == 2]), VOCABS[2])
    b3 = _greedy_ranges(np.sort(floc[fcl == 3]), VOCABS[3])
    bdefs = [(0, 0, VOCABS[0]), (1, 0, VOCABS[1])]
    bdefs += [(2, lo, hi) for lo, hi in b2]
    bdefs += [(3, lo, hi) for lo, hi in b3]
    bdefs = tuple(bdefs)
    nbk = len(bdefs)

    lo2 = np.asarray([lo for lo, hi in b2], dtype=np.int64)
    lo3 = np.asarray([lo for lo, hi in b3], dtype=np.int64)
    fb = np.where(
        fcl == 0, 0,
        np.where(
            fcl == 1, 1,
            np.where(
                fcl == 2,
                2 + np.searchsorted(lo2, floc, side="right") - 1,
                2 + len(b2) + np.searchsorted(lo3, floc, side="right") - 1,
            ),
        ),
    )
    order_g = np.argsort(fb, kind="stable")
    nbt = np.bincount(fb, minlength=nbk)  # global tokens per bucket
    starts = np.concatenate([[0], np.cumsum(nbt)])
    caps = tuple(
        int(max(1, -(-(-(-int(nbt[b]) // N_CORES)) // P))) for b in range(nbk)
    )

    cfg = _cfg()
    key = (bdefs, caps, tuple(sorted(cfg.items())))
    if key not in _BUILD_CACHE:
        _BUILD_CACHE[key] = _build(bdefs, caps, cfg)
    nc = _BUILD_CACHE[key]

    # per-core index uploads: bucket tokens dealt round-robin by global order
    in_maps = []
    for c in range(N_CORES):
        m = {f"emb{i}": embs[i] for i in range(4)}
        m["wts"] = wts
        cols = []
        for b in range(nbk):
            gidx = order_g[starts[b] : starts[b + 1]][c::N_CORES]
            seg = (floc[gidx] - bdefs[b][1]).astype(np.int16)
            padded = np.zeros(caps[b] * P, np.int16)
            padded[: len(seg)] = seg
            # dma_gather layout: index k at [k % 16, k // 16]; the 16-row
            # block replicated across partition groups (tx/rx Q7 cpu pairs
            # each stream their own 16-partition window)
            cols.append(np.tile(padded.reshape(caps[b] * 8, 16).T, (8, 1)))
        m["idx16"] = np.ascontiguousarray(np.concatenate(cols, axis=1))
        in_maps.append(m)

    res = run_bass_kernel_spmd(nc, in_maps, core_ids=list(range(N_CORES)))
    LAST_RESULT = res

    ctiles = [0] * 4
    tbase = []
    for b in range(nbk):
        tbase.append(ctiles[bdefs[b][0]])
        ctiles[bdefs[b][0]] += caps[b]

    flat = np.empty((N_CORES * ns, D_PROJ), np.float32)
    for c in range(N_CORES):
        unp = []
        for i in range(4):
            o = np.asarray(res.results[c][f"out{i}"])  # [128, ctiles_i, 1024]
            unp.append(o.transpose(1, 0, 2).reshape(ctiles[i] * P, D_PROJ))
        for b in range(nbk):
            gidx = order_g[starts[b] : starts[b + 1]][c::N_CORES]
            if len(gidx) == 0:
                continue
            i = bdefs[b][0]
            r0 = tbase[b] * P
            flat[gidx] = unp[i][r0 : r0 + len(gidx)].astype(np.float32)
    return flat.reshape(N_CORES, ns, D_PROJ)
